# revision 2
# baseline (speedup 1.0000x reference)
"""Linear-chain CRF loss on 8 Trainium2 cores — chunked-parallel forward.

The forward recursion alpha_t = LSE_i(alpha_{t-1} + T) + e_t is a strongly
contracting map, so each sequence is split into C = S/K chunks processed as
independent exp-space vector chains, each warm-started W steps early from a
uniform state. After W steps the chain's *direction* matches the true forward
state to ~1e-7; per-chunk log-magnitude growth telescopes into the exact
partition function. This turns 511 serial steps into Ns = K + W slots with
B/8 * C parallel chains per core.

Device layout (per core): chains ch = c*8 + b are columns; 4 row-groups of 32
states are stacked on the 128 SBUF partitions (chain ch -> row-group ch//CHq,
column ch%CHq). Each slot: 4 concurrent 32x32 matmuls (tile_position diagonal
quadrants, shared exp(T) weights) + one DVE multiply per half-group with the
pre-scaled emission block. Emissions are pre-exponentiated and pre-scaled on
the host by a predicted growth factor (LSE of e_t + log-colsum(expT)), so no
runtime rescaling is needed; everything is bf16 (state magnitudes stay within
e^{+-6} of 1).

Host does: gather/exp/prescale of emissions, the telescoped combine over the
state history, and the exact gold-path (labeled) score.
"""

import numpy as np

START_IDX = 29
END_IDX = 30
PAD_IDX = 31

B, S, L = 64, 512, 32
NCORES = 8
BPC = B // NCORES          # sequences per core

K = 4                      # chunk length
W = 1                      # warm-up steps (slot 0 of the warm-up runs on host)
C = S // K                 # chunks per sequence
NS = K + W                 # logical slots (slot 0 is computed on the host)
NSD = NS - 1               # device slots (1..NS-1)
CH = BPC * C               # chains per core
CHQ = CH // 4              # chains per 32-row group
HALF = CHQ // 2            # chains per half-group (pipeline group)

_nc = None


def set_config(k, w):
    """Re-derive the layout constants for a new (K, W). For tuning only."""
    global K, W, C, NS, NSD, CH, CHQ, HALF, _nc
    K, W = k, w
    C = S // K
    NS = K + W
    NSD = NS - 1
    CH = BPC * C
    CHQ = CH // 4
    HALF = CHQ // 2
    _nc = None


def _build_nc():
    import concourse.bacc as bacc
    import concourse.bass as bass
    import concourse.mybir as mybir
    from concourse import tile

    bf = mybir.dt.bfloat16
    f32 = mybir.dt.float32
    nc = bacc.Bacc(None, target_bir_lowering=False)

    # packed SBUF/dram layout (device runs logical slots 1..NS-1; slot 0's
    # state x0 is host-computed and shipped as the initial P block):
    #   input  = [ expT weights (L) | x0 (CHQ) | E slots 1..NS-1 (NSD*CHQ) ]
    #   sbuf   = input ++ [ P blocks for slots 1..NS-1 (NSD*CHQ) ]
    INP = L + CHQ + NSD * CHQ
    TOT = INP + NSD * CHQ
    inp = nc.declare_dram_parameter("inp", (128, INP), bf, isOutput=False)
    p_out = nc.declare_dram_parameter("p_all", (128, NSD * CHQ), bf, isOutput=True)

    E0 = L + CHQ

    def pblk(i):  # column offset of P block for device step i (0 = x0)
        return L if i == 0 else INP + (i - 1) * CHQ

    # output chunking: small trailing chunk so the tail is short
    out_splits = sorted(x for x in {NSD - 3, NSD - 1, NSD} if x > 0)

    with tile.TileContext(nc) as tc:
        with (
            tc.tile_pool(name="big", bufs=1) as big,
            tc.tile_pool(name="qp", bufs=4, space=bass.MemorySpace.PSUM) as qp,
        ):
            ALL = big.tile([128, TOT], bf)
            WT = ALL[:, 0:L]

            # input DMAs fan out across engine queues so their fixed issue +
            # DGE + sem-propagation latencies overlap: the critical first
            # chunk [wt|x0] goes on Pool; per-slot E blocks alternate between
            # the otherwise-idle SP and Activation queues.
            nc.gpsimd.dma_start(ALL[:, 0:E0], inp[:, 0:E0])
            for i in range(1, NSD + 1):
                lo = E0 + (i - 1) * CHQ
                hi = E0 + i * CHQ
                eng = nc.sync if i % 2 == 1 else nc.scalar
                eng.dma_start(ALL[:, lo:hi], inp[:, lo:hi])

            osrc = 0
            for i in range(1, NSD + 1):
                for h in range(2):
                    c0 = E0 + (i - 1) * CHQ + h * HALF
                    p_prev = pblk(i - 1) + h * HALF
                    p_next = pblk(i) + h * HALF
                    q = qp.tile([128, HALF], f32, tag=f"q{h}")
                    for rg in range(4):
                        r = 32 * rg
                        nc.tensor.matmul(
                            q[r:r + 32, :], WT[r:r + 32, :],
                            ALL[r:r + 32, p_prev:p_prev + HALF],
                            start=True, stop=True,
                            tile_position=(r, r),
                        )
                    nc.vector.tensor_mul(ALL[:, p_next:p_next + HALF], q[:],
                                         ALL[:, c0:c0 + HALF])
                if i in out_splits:
                    nc.gpsimd.dma_start(
                        p_out[:, osrc * CHQ:i * CHQ],
                        ALL[:, pblk(osrc + 1):pblk(osrc + 1) + (i - osrc) * CHQ])
                    osrc = i

    nc.compile()
    return nc


def _lse(x, axis=-1):
    m = np.max(x, axis=axis, keepdims=True)
    return (m + np.log(np.sum(np.exp(x - m), axis=axis, keepdims=True))).squeeze(axis)


def _host_prep(lstm, T):
    """Eg [B, C, NSD, L] bf16 (slots 1..NS-1), x0 [B, C, L] bf16 (slot-0 state,
    host-computed), gsum [B, C, NS] f64, offs [C], p0shift [B]."""
    import ml_dtypes
    BF = ml_dtypes.bfloat16
    offs = np.array([1 if c == 0 else max(1, c * K - W) for c in range(C)])
    expT = np.exp(T.astype(np.float64))
    with np.errstate(divide="ignore"):
        lcs = np.log(expT.sum(axis=0))                    # [L]
    tt = offs[:, None] + np.arange(NS)[None, :]           # [C, NS]
    e = lstm[:, tt, :].astype(np.float64)                 # [B, C, NS, L]
    ghat = _lse(e + lcs[None, None, None, :], axis=3) - np.log(L)
    E_all = np.exp(e - ghat[..., None])
    gsum = np.cumsum(ghat, axis=2)
    # slot-0 state on host: x0 = E_0 * (expT^T @ init)
    p0 = T[START_IDX, :][None, :] + lstm[:, 0, :].astype(np.float64)
    p0shift = p0.max(axis=1)
    q0 = np.broadcast_to(expT.sum(axis=0), (B, C, L)).copy()   # init = ones
    q0[:, 0, :] = np.exp(p0 - p0shift[:, None]) @ expT         # init = p0
    x0 = (E_all[:, :, 0, :] * q0).astype(BF)
    Eg = E_all[:, :, 1:, :].astype(BF)
    return Eg, x0, gsum, offs, p0shift


def _device_arrays(Eg, x0, T):
    """Per-core input maps. Chain ch = c*BPC + b -> row-group ch//CHQ, col ch%CHQ."""
    import ml_dtypes
    BF = ml_dtypes.bfloat16
    expT = np.exp(T.astype(np.float64)).astype(BF)        # [L, L]
    wt = np.tile(expT, (4, 1))                            # [128, L]
    in_maps = []
    for core in range(NCORES):
        sl = slice(core * BPC, (core + 1) * BPC)
        Ec = Eg[sl]                                       # [BPC, C, NSD, L]
        # chain index ch = c*BPC + b; value at [128 part, s*CHQ + q]
        # part = 32*rg + j ; rg = ch // CHQ ; q = ch % CHQ
        Ech = Ec.transpose(1, 0, 2, 3).reshape(CH, NSD, L)  # [ch, s, j]
        Em = Ech.reshape(4, CHQ, NSD, L).transpose(0, 3, 2, 1).reshape(128, NSD * CHQ)
        Ic = x0[sl].transpose(1, 0, 2).reshape(CH, L)
        Im = Ic.reshape(4, CHQ, L).transpose(0, 2, 1).reshape(128, CHQ)
        packed = np.concatenate([wt, Im, Em], axis=1)     # [128, L+CHQ+NSD*CHQ]
        in_maps.append({"inp": np.ascontiguousarray(packed)})
    return in_maps


def _unlabeled_from_hist(hists, x0, gsum, p0shift, lens, T):
    """hists: per-core p_all [128, NSD*CHQ] bf16 (slots 1..NS-1); x0 [B, C, L]
    is the host-computed slot-0 state. Fully vectorized."""
    hs = []
    for core in range(NCORES):
        h = np.asarray(hists[core], dtype=np.float64).reshape(4, L, NSD, CHQ)
        h = h.transpose(2, 1, 0, 3).reshape(NSD, L, CH)   # [s, j, ch]
        h = h.reshape(NSD, L, C, BPC)                     # ch = c*BPC + b
        hs.append(h)
    hist = np.concatenate([h.transpose(0, 1, 3, 2) for h in hs], axis=2)
    # [NSD, L, B, C]; prepend host slot 0 -> [NS, L, B, C]
    hist = np.concatenate([x0.astype(np.float64).transpose(2, 0, 1)[None],
                           hist], axis=0)

    colsum = hist.sum(axis=1)                             # [NS, B, C]
    with np.errstate(divide="ignore"):
        logmag = np.log(np.maximum(colsum, 1e-300)) + gsum.transpose(2, 0, 1)
    # logmag[s, b, c] now absolute (up to p0shift for c=0)
    logmag[:, :, 0] += p0shift[None, :]

    offs = np.array([1 if c == 0 else max(1, c * K - W) for c in range(C)])
    lens = lens.astype(np.int64)
    cstar = (lens - 1) // K                               # [B]
    ostar = (lens - 1) - cstar * K

    cc = np.arange(C)
    warm_slot = cc * K - 1 - offs                         # valid for c>=1
    full_end_slot = cc * K + (K - 1) - offs               # c>=1
    # full-chunk growths for c >= 1
    bidx = np.arange(B)
    g_full = (logmag[full_end_slot[1:], :, cc[1:]] -
              logmag[warm_slot[1:], :, cc[1:]])           # [C-1, B]
    g_pref = np.concatenate([np.zeros((1, B)), np.cumsum(g_full, axis=0)], axis=0)
    # g_pref[k, b] = sum of full growths for chunks 1..k

    # partial growth of chunk cstar (to ostar); for cstar=0 handled separately
    end_slot_star = cstar * K + ostar - offs[cstar]       # [B]
    w_end = T[:, END_IDX].astype(np.float64)

    U = np.zeros(B)
    for b in range(B):
        length = int(lens[b])
        cs, os_ = int(cstar[b]), int(ostar[b])
        if length == 1:
            p0 = T[START_IDX, :].astype(np.float64) + 0.0
            U[b] = _lse(p0 + w_end)  # lstm added by caller path below
            continue
        if cs == 0:
            slot = os_ - 1
            x = np.log(np.maximum(hist[slot, :, b, 0], 1e-300))
            U[b] = gsum[b, 0, slot] + p0shift[b] + _lse(x + w_end)
            continue
        M0 = logmag[K - 2, b, 0]
        Gmid = g_pref[cs - 1, b]                          # chunks 1..cs-1
        es = int(end_slot_star[b])
        g_star = logmag[es, b, cs] - logmag[warm_slot[cs], b, cs]
        x = np.log(np.maximum(hist[es, :, b, cs], 1e-300))
        d = x - np.log(np.maximum(colsum[es, b, cs], 1e-300))
        U[b] = M0 + Gmid + g_star + _lse(d + w_end)
    return U


def _labeled_score(lstm_scores, word_seq_lens, tags, mask, transition):
    b_idx = np.arange(B)
    t0 = tags[:, 0]
    begin = transition[START_IDX, t0].astype(np.float64) + lstm_scores[b_idx, 0, t0]
    prev, curt = tags[:, :-1], tags[:, 1:]
    trans_mid = transition[prev, curt].astype(np.float64)
    em_mid = np.take_along_axis(lstm_scores[:, 1:, :], curt[..., None], axis=2)[..., 0]
    mid = np.where(mask[:, 1:], trans_mid + em_mid, 0.0)
    end_ids = tags[b_idx, word_seq_lens - 1]
    end_sc = transition[end_ids, END_IDX].astype(np.float64)
    return begin.sum() + end_sc.sum() + mid.sum()


_exec = None


def _run_cached(in_maps):
    """Like bass2jax.run_bass_via_pjrt but with the jitted executable cached
    across calls, so repeat invocations skip JAX retracing."""
    global _exec
    import jax
    import numpy as np_
    from jax.sharding import Mesh, PartitionSpec
    from jax.experimental.shard_map import shard_map
    from concourse import bass2jax

    if _exec is None:
        bass2jax.install_neuronx_cc_hook()
        nc = _nc
        out_name = "p_all"
        out_shape = (128, NSD * CHQ)
        import ml_dtypes
        out_dtype = ml_dtypes.bfloat16

        part_name = nc.partition_id_tensor.name if nc.partition_id_tensor else None
        in_names = ["inp", out_name] + ([part_name] if part_name else [])

        def _body(inp_arr, zero_out):
            operands = [inp_arr, zero_out]
            if part_name:
                operands.append(bass2jax.partition_id_tensor())
            outs = bass2jax._bass_exec_p.bind(
                *operands,
                out_avals=(jax.core.ShapedArray(out_shape, out_dtype),),
                in_names=tuple(in_names),
                out_names=(out_name,),
                lowering_input_output_aliases=(),
                sim_require_finite=True,
                sim_require_nnan=True,
                nc=nc,
            )
            return tuple(outs)

        devices = jax.devices()[:NCORES]
        mesh = Mesh(np_.asarray(devices), ("core",))
        sharded = jax.jit(
            shard_map(_body, mesh=mesh,
                      in_specs=(PartitionSpec("core"),) * 2,
                      out_specs=(PartitionSpec("core"),),
                      check_rep=False),
            donate_argnums=(1,), keep_unused=True,
        )
        _exec = (sharded, out_shape, out_dtype)

    sharded, out_shape, out_dtype = _exec
    concat_in = np.concatenate([m["inp"] for m in in_maps], axis=0)
    zeros = np.zeros((NCORES * out_shape[0], out_shape[1]), out_dtype)
    out = sharded(concat_in, zeros)[0]
    out = np.asarray(out).reshape(NCORES, *out_shape)
    return [out[c] for c in range(NCORES)]


def kernel(lstm_scores, word_seq_lens, tags, mask, transition):
    global _nc
    lstm_scores = np.asarray(lstm_scores, dtype=np.float32)
    word_seq_lens = np.asarray(word_seq_lens).astype(np.int64)
    tags = np.asarray(tags).astype(np.int64)
    mask = np.asarray(mask).astype(bool)
    transition = np.asarray(transition, dtype=np.float32)

    if _nc is None:
        _nc = _build_nc()

    Eg, x0, gsum, offs, p0shift = _host_prep(lstm_scores, transition)
    in_maps = _device_arrays(Eg, x0, transition)

    hists = _run_cached(in_maps)

    U = _unlabeled_from_hist(hists, x0, gsum, p0shift, word_seq_lens, transition)
    # len==1 sequences need the emission term added (kernel never ran them)
    for b in np.nonzero(word_seq_lens == 1)[0]:
        p0 = transition[START_IDX, :].astype(np.float64) + lstm_scores[b, 0, :]
        U[b] = _lse(p0 + transition[:, END_IDX].astype(np.float64))
    unlabeled = U.sum()

    labeled = _labeled_score(lstm_scores, word_seq_lens, tags, mask, transition)
    return (np.float32(unlabeled), np.float32(labeled))
